# revision 5
# baseline (speedup 1.0000x reference)
"""Trainium2 Bass kernel for nn_BiARMA (2-layer ARMA GNN, K=2 stacks, T=2).

Math: A = D^-1/2 C D^-1/2 (C = edge-count matrix, deg by dst).
Key identity: norm[e] = dinv[src]*dinv[dst] factors, so
  segment_sum(out[src]*norm, dst) = dinv_dst * gather_sum(dinv_src*out[src])
-> every message-passing round is a pure row-gather-accumulate of a
pre-scaled node tensor.  Weights commute with aggregation, so matmuls
run on the aggregated tensor.

Distribution: dst-nodes sharded over 8 cores (graph parallel).  Each
core keeps a full replica of the current pre-scaled node tensor in its
DRAM, split into two halves by LOCAL node index (every core's locals
[0,NHALF) form half A, [NHALF,NPC) half B) so int16 gather indices
reach every row.  Each round runs in two waves:
  wave A: dma_gather (prepare_only + trigger) the half-A slot columns
          of every group from yA, reduce them into a persistent
          per-block accumulator (depends only on AG_A of the previous
          round's output);
  wave B: gather half-B columns from yB, reduce, add the accumulator,
          apply weights on PE, store the output shard.
The output shard's A half (blocks < NB/2) is AllGathered as soon as
wave B passes the midpoint, so AG_A overlaps the rest of wave B and
AG_B overlaps the next round's wave A.  Round 2 (the K*HID=128-wide
round, 2x the volume of the others) carries its payload in bf16,
halving both its gather and AllGather traffic.
"""

import os
import sys
from dataclasses import dataclass, field

import numpy as np

sys.path.insert(0, "/opt/trn_rl_repo")

P = 128


@dataclass
class Cfg:
    N: int = 50000
    E: int = 800000
    IN_C: int = 64
    HID_C: int = 64
    OUT_C: int = 32
    K: int = 2
    CORES: int = 8
    # gather-tile budget, BYTES per partition per wave-call
    group_budget_bytes: int = 32768
    # per-call column cap: cols*128 descriptors must stay well under the
    # 16384-descriptor SWDGE ring
    col_cap: int = 120

    @property
    def blocks(self):
        # even block count so NPC/2 is a whole number of blocks; the
        # spare rows guarantee phantom pad rows in each half
        nb = (self.N // self.CORES + 2 + P - 1) // P
        return nb + (nb & 1)

    @property
    def NPC(self):
        return self.blocks * P

    @property
    def NBH(self):  # blocks in half A
        return self.blocks // 2

    @property
    def NHALF(self):  # local rows per half
        return self.NPC // 2

    @property
    def RH(self):  # rows per half replica (all cores' halves stacked)
        return self.CORES * self.NHALF

    @property
    def PAD_LOC(self):  # phantom row (value 0) inside each half replica
        return self.NHALF - 1


@dataclass
class Struct:
    DA: list
    DB: list
    col_off: list
    a_off: list
    b_off: list
    tot_cols: int
    idx16: np.ndarray
    idx32: np.ndarray
    pid: np.ndarray
    a_cum: list = None
    b_cum: list = None
    TA: int = 0
    TB: int = 0
    groups: dict = field(default_factory=dict)


def build_structure(edge_index: np.ndarray, cfg: Cfg) -> Struct:
    src = np.asarray(edge_index[0], dtype=np.int64)
    dst = np.asarray(edge_index[1], dtype=np.int64)
    N, CORES, NPC, NB = cfg.N, cfg.CORES, cfg.NPC, cfg.blocks
    NHALF = cfg.NHALF

    deg = np.bincount(dst, minlength=N)
    order = np.argsort(-deg, kind="stable")
    rank = np.arange(N)
    core_of = np.empty(N, np.int64)
    raw_local = np.empty(N, np.int64)
    core_of[order] = rank % CORES
    raw_local[order] = rank // CORES
    # skip local NHALF-1 so each half keeps a phantom (zero) pad row
    local_of = raw_local + (raw_local >= (NHALF - 1))
    assert local_of.max() < NPC - 1
    pid = core_of * NPC + local_of

    ecore = core_of[dst]
    dloc = local_of[dst]
    half = (local_of[src] >= NHALF).astype(np.int64)
    # row within the half replica [RH]
    sloc = core_of[src] * NHALF + (local_of[src] - half * NHALF)

    # per (core, node, half) counts -> per-block padded A/B widths
    cnt = np.zeros((2, CORES, NPC), np.int64)
    for h in (0, 1):
        for c in range(CORES):
            m = (ecore == c) & (half == h)
            cnt[h, c] = np.bincount(dloc[m], minlength=NPC)
    DA = cnt[0].reshape(CORES, NB, P).max(axis=(0, 2))
    DB = cnt[1].reshape(CORES, NB, P).max(axis=(0, 2))
    DA = np.maximum(DA, 1).tolist()
    DB = np.maximum(DB, 1).tolist()
    D = [DA[b] + DB[b] for b in range(NB)]
    col_off = np.concatenate([[0], np.cumsum(D)]).tolist()
    a_off = [col_off[b] for b in range(NB)]
    b_off = [col_off[b] + DA[b] for b in range(NB)]
    tot_cols = int(col_off[-1])

    # per-slot values, node-major layout [P, tot_cols]
    vals = np.full((CORES, P, tot_cols), -1, np.int64)
    eo = np.lexsort((dloc, ecore))
    ecore_s, dloc_s, sloc_s, half_s = ecore[eo], dloc[eo], sloc[eo], half[eo]
    aoff = np.asarray(a_off)
    boff = np.asarray(b_off)
    for c in range(CORES):
        m = ecore_s == c
        dl, sl, hh = dloc_s[m], sloc_s[m], half_s[m]
        for h in (0, 1):
            mh = hh == h
            dlh, slh = dl[mh], sl[mh]
            cth = np.bincount(dlh, minlength=NPC)
            starts = np.concatenate([[0], np.cumsum(cth)])[:-1]
            pos = np.arange(dlh.shape[0]) - starts[dlh]
            b = dlh // P
            p = dlh % P
            col = (aoff if h == 0 else boff)[b] + pos
            vals[c, p, col] = slh

    idx32 = vals.astype(np.int32)  # -1 = pad (degree helper)

    v16 = np.where(vals < 0, cfg.PAD_LOC, vals).astype(np.int16)

    st = Struct(DA=DA, DB=DB, col_off=col_off, a_off=a_off, b_off=b_off,
                tot_cols=tot_cols, idx16=None, idx32=idx32, pid=pid)

    # bytes moved per gathered row, per round
    G1 = cfg.K * cfg.HID_C
    G2 = cfg.K * cfg.OUT_C
    row_bytes = {1: cfg.IN_C * 4, 2: G1 * 2, 3: cfg.HID_C * 4, 4: G2 * 4}

    def make_groups(rb):
        budget = max(min(cfg.group_budget_bytes // rb, cfg.col_cap), max(D))
        groups = []
        for lo, hi in ((0, cfg.NBH), (cfg.NBH, NB)):
            b0 = lo
            while b0 < hi:
                b1 = b0
                tot = 0
                while b1 < hi and (tot + D[b1] <= budget or b1 == b0):
                    tot += D[b1]
                    b1 += 1
                groups.append((b0, b1))
                b0 = b1
        return groups

    st.groups = {r: make_groups(row_bytes[r]) for r in (1, 2, 3, 4)}

    # idx16 storage: all A-columns (block-major), then all B-columns,
    # each wrapped for the dma_gather call layout.
    a_cum = np.concatenate([[0], np.cumsum(DA)]).astype(np.int64)
    b_cum = np.concatenate([[0], np.cumsum(DB)]).astype(np.int64)
    TA, TB = int(a_cum[-1]), int(b_cum[-1])
    st.a_cum = a_cum.tolist()
    st.b_cum = b_cum.tolist()
    st.TA, st.TB = TA, TB

    idx16 = np.empty((CORES, 16, (TA + TB) * 8), np.int16)
    for c in range(CORES):
        va = np.empty((P, TA), np.int16)
        vb = np.empty((P, TB), np.int16)
        for b in range(NB):
            va[:, a_cum[b]:a_cum[b + 1]] = \
                v16[c][:, a_off[b]:a_off[b] + DA[b]]
            vb[:, b_cum[b]:b_cum[b + 1]] = \
                v16[c][:, b_off[b]:b_off[b] + DB[b]]
        both = np.concatenate([va, vb], axis=1)  # [P, TA+TB]
        # column c', partition p -> flat i = c'*128 + p -> (i%16, i//16)
        w = both.reshape(16, 8, TA + TB, order="F")
        w2 = np.transpose(w, (0, 2, 1)).reshape(16, (TA + TB) * 8)
        idx16[c] = w2
    st.idx16 = np.tile(idx16, (1, 8, 1))  # replicate per Q7 core
    return st


def build_weight_inputs(inp: dict, cfg: Cfg) -> dict:
    K, IN_C, HID_C, OUT_C = cfg.K, cfg.IN_C, cfg.HID_C, cfg.OUT_C
    f4 = lambda a: np.ascontiguousarray(a, dtype=np.float32)

    rootw1 = np.transpose(inp["root_w1"][0], (1, 0, 2)).reshape(IN_C, K * HID_C)
    b1row = inp["b1"][0, :, 0, :].reshape(1, K * HID_C)
    initw1 = np.transpose(inp["init_w1"], (1, 0, 2)).reshape(IN_C, K * HID_C)
    w1bd = np.zeros((K * HID_C, K * HID_C), np.float32)
    for k in range(K):
        w1bd[k * HID_C:(k + 1) * HID_C, k * HID_C:(k + 1) * HID_C] = inp["w1"][0, k]

    # 0.5 absorbed: round-2 h-stage feeds the UNhalved stack sum into root2
    rootw2 = 0.5 * np.transpose(inp["root_w2"][0], (1, 0, 2)).reshape(HID_C, K * OUT_C)
    b2row = inp["b2"][0, :, 0, :].reshape(1, K * OUT_C)
    initw2 = np.transpose(inp["init_w2"], (1, 0, 2)).reshape(HID_C, K * OUT_C)
    w2bd = np.zeros((K * OUT_C, K * OUT_C), np.float32)
    for k in range(K):
        w2bd[k * OUT_C:(k + 1) * OUT_C, k * OUT_C:(k + 1) * OUT_C] = inp["w2"][0, k]

    return {
        "w_rootw1": f4(rootw1), "w_b1": f4(b1row), "w_initw1": f4(initw1),
        "w_w1bd": f4(w1bd), "w_rootw2": f4(rootw2), "w_b2": f4(b2row),
        "w_initw2": f4(initw2), "w_w2bd": f4(w2bd),
    }


def build_nc(cfg: Cfg, st: Struct):
    import concourse.bacc as bacc
    import concourse.mybir as mybir
    import concourse.tile as tile
    from concourse.masks import make_identity

    f32 = mybir.dt.float32
    bf16 = mybir.dt.bfloat16
    i16 = mybir.dt.int16
    i32 = mybir.dt.int32
    X = mybir.AxisListType.X
    Alu = mybir.AluOpType
    Act = mybir.ActivationFunctionType

    K, IN_C, HID_C, OUT_C = cfg.K, cfg.IN_C, cfg.HID_C, cfg.OUT_C
    G1 = K * HID_C   # 128
    G2 = K * OUT_C   # 64
    NB = cfg.blocks
    NBH = cfg.NBH
    NPC, NHALF, RH = cfg.NPC, cfg.NHALF, cfg.RH
    DA, DB = st.DA, st.DB
    TA, TB = st.TA, st.TB
    a_cum, b_cum = st.a_cum, st.b_cum
    WTOT = (TA + TB) * 8

    FW = {1: IN_C, 2: G1, 3: HID_C, 4: G2}   # gathered row width (elems)
    GW = {1: G1, 2: G1, 3: G2, 4: G2}        # matmul output width
    YDT = {1: f32, 2: bf16, 3: f32, 4: f32}  # payload dtype per round
    DSZ = {1: 4, 2: 2, 3: 4, 4: 4}

    nc = bacc.Bacc(
        "TRN2",
        target_bir_lowering=False,
        debug=False,
        num_devices=cfg.CORES,
    )

    # ---- kernel I/O ----
    xs = nc.dram_tensor("xs", [NPC, IN_C], f32, kind="ExternalInput")
    idx16_d = nc.dram_tensor("idx16", [P, WTOT], i16, kind="ExternalInput")
    idx32_d = nc.dram_tensor("idx32", [P, st.tot_cols], i32, kind="ExternalInput")
    w_rootw1 = nc.dram_tensor("w_rootw1", [IN_C, G1], f32, kind="ExternalInput")
    w_b1 = nc.dram_tensor("w_b1", [1, G1], f32, kind="ExternalInput")
    w_initw1 = nc.dram_tensor("w_initw1", [IN_C, G1], f32, kind="ExternalInput")
    w_w1bd = nc.dram_tensor("w_w1bd", [G1, G1], f32, kind="ExternalInput")
    w_rootw2 = nc.dram_tensor("w_rootw2", [HID_C, G2], f32, kind="ExternalInput")
    w_b2 = nc.dram_tensor("w_b2", [1, G2], f32, kind="ExternalInput")
    w_initw2 = nc.dram_tensor("w_initw2", [HID_C, G2], f32, kind="ExternalInput")
    w_w2bd = nc.dram_tensor("w_w2bd", [G2, G2], f32, kind="ExternalInput")
    out_d = nc.dram_tensor("out", [NPC, OUT_C], f32, kind="ExternalOutput")

    # ---- internal DRAM: per-half replicas + per-half AG inputs ----
    yA = {r: nc.dram_tensor(f"yA{r}", [RH, FW[r]], YDT[r], addr_space="Shared")
          for r in (1, 2, 3, 4)}
    yB = {r: nc.dram_tensor(f"yB{r}", [RH, FW[r]], YDT[r], addr_space="Shared")
          for r in (1, 2, 3, 4)}
    agA = {r: nc.dram_tensor(f"agA{r}", [NHALF, FW[r]], YDT[r])
           for r in (1, 2, 3, 4)}
    agB = {r: nc.dram_tensor(f"agB{r}", [NHALF, FW[r]], YDT[r])
           for r in (1, 2, 3, 4)}

    rg = [list(range(cfg.CORES))]

    # gather tile: sized in f32 elems, bitcast for bf16 rounds
    max_gt_f32 = 0
    for r in (1, 2, 3, 4):
        for (b0, b1) in st.groups[r]:
            nA = (a_cum[b1] - a_cum[b0]) * FW[r] * DSZ[r]
            nB = (b_cum[b1] - b_cum[b0]) * FW[r] * DSZ[r]
            max_gt_f32 = max(max_gt_f32, (max(nA, nB) + 3) // 4)

    dsem = nc.alloc_semaphore("gsem")
    NSEM = 8
    dsems = [nc.alloc_semaphore(f"gsem{i}") for i in range(NSEM)]
    sem_count = [0] * NSEM
    gidx = [0]

    max_round = int(os.environ.get("GNN_STAGE", "4"))
    skip_ag = bool(os.environ.get("GNN_SKIP_AG"))
    lite = bool(os.environ.get("GNN_R1_LITE"))
    tiny = bool(os.environ.get("GNN_TINY_GATHER"))

    with tile.TileContext(nc) as tc:
        with (
            tc.tile_pool(name="const", bufs=1) as cpool,
            tc.tile_pool(name="gather", bufs=3) as gpool,
            tc.tile_pool(name="work", bufs=3) as wpool,
            tc.tile_pool(name="psum", bufs=3, space="PSUM") as ppool,
        ):
            # ---------- constants ----------
            ident = cpool.tile([P, P], f32)
            make_identity(nc, ident[:])

            def load_w(t, shape, tag):
                s = cpool.tile(list(shape), f32, tag=tag)
                nc.sync.dma_start(out=s[:], in_=t[:, :])
                return s

            rootw1_s = load_w(w_rootw1, (IN_C, G1), "w_rootw1")
            initw1_s = load_w(w_initw1, (IN_C, G1), "w_initw1")
            w1bd_s = load_w(w_w1bd, (G1, G1), "w_w1bd")
            rootw2_s = load_w(w_rootw2, (HID_C, G2), "w_rootw2")
            initw2_s = load_w(w_initw2, (HID_C, G2), "w_initw2")
            w2bd_s = load_w(w_w2bd, (G2, G2), "w_w2bd")
            b1_s = load_w(w_b1, (1, G1), "w_b1")
            b2_s = load_w(w_b2, (1, G2), "w_b2")
            rhs_s = {1: initw1_s, 2: w1bd_s, 3: initw2_s, 4: w2bd_s}

            ones1 = cpool.tile([1, P], f32)
            nc.vector.memset(ones1[:], 1.0)
            b1rep = cpool.tile([P, G1], f32)
            b2rep = cpool.tile([P, G2], f32)
            bps = ppool.tile([P, G1], f32, tag="mmps")
            nc.tensor.matmul(bps[:], lhsT=ones1[:], rhs=b1_s[:], start=True, stop=True)
            nc.vector.tensor_copy(b1rep[:], bps[:])
            bps2 = ppool.tile([P, G2], f32, tag="mmps")
            nc.tensor.matmul(bps2[:], lhsT=ones1[:], rhs=b2_s[:], start=True, stop=True)
            nc.vector.tensor_copy(b2rep[:], bps2[:])

            # ---------- gather indices ----------
            idx16_s = cpool.tile([P, WTOT], i16)
            nc.sync.dma_start(out=idx16_s[:], in_=idx16_d[:, :])

            # ---------- persistent per-round state ----------
            root1 = cpool.tile([P, NB, G1], f32)
            root2 = cpool.tile([P, NB, G2], f32)
            aggAcc = cpool.tile([P, NB, G1], f32)  # wave-A partial sums
            dinv = cpool.tile([P, NB], f32)
            dinvh = cpool.tile([P, NB], f32)

            # ---------- degrees + roots + y1 (prolog) ----------
            with tc.tile_pool(name="prolog", bufs=1) as qpool:
                idx32_s = qpool.tile([P, st.tot_cols], i32)
                nc.sync.dma_start(out=idx32_s[:], in_=idx32_d[:, :])
                idxf = qpool.tile([P, st.tot_cols], f32)
                nc.vector.tensor_copy(idxf[:], idx32_s[:])
                valid = qpool.tile([P, st.tot_cols], f32)
                nc.vector.tensor_single_scalar(
                    valid[:], idxf[:], -1.0, Alu.not_equal
                )
                deg = qpool.tile([P, NB], f32)
                for b in range(NB):
                    c0, c1 = st.col_off[b], st.col_off[b + 1]
                    nc.vector.reduce_sum(deg[:, b:b + 1], valid[:, c0:c1], axis=X)
                degc = qpool.tile([P, NB], f32)
                nc.vector.tensor_scalar_max(degc[:], deg[:], 1.0)
                sq = qpool.tile([P, NB], f32)
                nc.scalar.activation(sq[:], degc[:], Act.Sqrt)
                rinv = qpool.tile([P, NB], f32)
                nc.vector.reciprocal(rinv[:], sq[:])
                mask = qpool.tile([P, NB], f32)
                nc.vector.tensor_single_scalar(mask[:], deg[:], 0.0, Alu.is_gt)
                nc.vector.tensor_mul(dinv[:], rinv[:], mask[:])
                nc.vector.tensor_scalar_mul(dinvh[:], dinv[:], 0.5)

                x_s = qpool.tile([P, NB, IN_C], f32)
                for b in range(NB):
                    nc.sync.dma_start(
                        out=x_s[:, b, :], in_=xs[b * P:(b + 1) * P, :]
                    )
                for b in range(NB):
                    dcol = dinv[:, b:b + 1]
                    xT_ps = ppool.tile([IN_C, P], f32, tag="tps")
                    nc.tensor.transpose(xT_ps[:], x_s[:, b, :], ident[:])
                    xT = wpool.tile([IN_C, P], f32, tag="aggT")
                    nc.scalar.activation(xT[:], xT_ps[:], Act.Copy)
                    r1_ps = ppool.tile([P, G1], f32, tag="mmps")
                    nc.tensor.matmul(
                        r1_ps[:], lhsT=xT[:], rhs=rootw1_s[:], start=True, stop=True
                    )
                    nc.vector.tensor_add(root1[:, b, :], r1_ps[:], b1rep[:])
                    y1b = wpool.tile([P, IN_C], f32, tag="yout")
                    nc.scalar.activation(y1b[:], x_s[:, b, :], Act.Copy, scale=dcol)
                    if b < NBH:
                        nc.sync.dma_start(
                            out=agA[1][b * P:(b + 1) * P, :], in_=y1b[:]
                        )
                    else:
                        bb = b - NBH
                        nc.sync.dma_start(
                            out=agB[1][bb * P:(bb + 1) * P, :], in_=y1b[:]
                        )
                    if b == NBH - 1 and not skip_ag:
                        nc.gpsimd.collective_compute(
                            "AllGather", Alu.bypass, replica_groups=rg,
                            ins=[agA[1].ap().opt()], outs=[yA[1].ap().opt()],
                        )
            if not skip_ag:
                nc.gpsimd.collective_compute(
                    "AllGather", Alu.bypass, replica_groups=rg,
                    ins=[agB[1].ap().opt()], outs=[yB[1].ap().opt()],
                )

            # ---------- gather helper ----------
            def gather_wave(r, b0, b1, wave):
                """prepare_only dma_gather of the A- or B-half columns of
                blocks [b0,b1); returns (tile, (sem, value))."""
                F = FW[r]
                dt = YDT[r]
                if wave == "A":
                    ncols = a_cum[b1] - a_cum[b0]
                    ix = idx16_s[:, a_cum[b0] * 8:a_cum[b1] * 8]
                    yv = yA[r]
                else:
                    ncols = b_cum[b1] - b_cum[b0]
                    ix = idx16_s[:, (TA + b_cum[b0]) * 8:(TA + b_cum[b1]) * 8]
                    yv = yB[r]
                n = ncols * P
                gt = gpool.tile([P, max_gt_f32], f32, tag="gt")
                gv = gt[:].bitcast(dt) if dt != f32 else gt[:]
                out = gv[:, :ncols * F].rearrange("p (c f) -> p c f", f=F)
                s = gidx[0] % NSEM
                gidx[0] += 1
                sem_count[s] += 16
                if tiny:
                    nc.gpsimd.dma_gather(
                        out_ap=gv[:, :F].rearrange("p (c f) -> p c f", f=F),
                        in_ap=yv.ap(), idxs_ap=idx16_s[:, :8],
                        num_idxs=P, num_idxs_reg=P, elem_size=F,
                        single_packet=False, prepare_only=True, sem=dsems[s],
                    )
                else:
                    nc.gpsimd.dma_gather(
                        out_ap=out, in_ap=yv.ap(), idxs_ap=ix,
                        num_idxs=n, num_idxs_reg=n, elem_size=F,
                        single_packet=False, prepare_only=True, sem=dsems[s],
                    )
                nc.gpsimd.trigger_dma(count=None)
                return gv, (dsems[s], sem_count[s])

            # ---------- 4 message-passing rounds ----------
            for r in (1, 2, 3, 4):
                if r > max_round:
                    break
                F = FW[r]
                G = GW[r]
                # ---- wave A: gather half-A columns, accumulate ----
                for (b0, b1) in st.groups[r]:
                    gv, (ws, wv) = gather_wave(r, b0, b1, "A")
                    if lite:
                        lt = wpool.tile([P, F], f32, tag="agg")
                        cp = nc.vector.tensor_copy(lt[:], gv[:, :F])
                        cp._wait_ge(ws, wv)
                        continue
                    for b in range(b0, b1):
                        oA = a_cum[b] - a_cum[b0]
                        rA = nc.vector.reduce_sum(
                            aggAcc[:, b, :F],
                            gv[:, oA * F:(oA + DA[b]) * F].rearrange(
                                "p (d f) -> p f d", f=F
                            ),
                            axis=X,
                        )
                        rA._wait_ge(ws, wv)
                # ---- wave B: gather half-B, finish blocks ----
                for (b0, b1) in st.groups[r]:
                    gv, (ws, wv) = gather_wave(r, b0, b1, "B")
                    if lite:
                        lt = wpool.tile([P, F], f32, tag="agg")
                        cp = nc.vector.tensor_copy(lt[:], gv[:, :F])
                        cp._wait_ge(ws, wv)
                        continue
                    for b in range(b0, b1):
                        dcol = dinv[:, b:b + 1]
                        oB = b_cum[b] - b_cum[b0]
                        aggB = wpool.tile([P, F], f32, tag="aggB")
                        rB = nc.vector.reduce_sum(
                            aggB[:],
                            gv[:, oB * F:(oB + DB[b]) * F].rearrange(
                                "p (d f) -> p f d", f=F
                            ),
                            axis=X,
                        )
                        rB._wait_ge(ws, wv)
                        agg = wpool.tile([P, F], f32, tag="agg")
                        nc.vector.tensor_add(agg[:], aggAcc[:, b, :F], aggB[:])
                        aggT_ps = ppool.tile([F, P], f32, tag="tps")
                        nc.tensor.transpose(aggT_ps[:], agg[:], ident[:])
                        aggT = wpool.tile([F, P], f32, tag="aggT")
                        nc.scalar.activation(aggT[:], aggT_ps[:], Act.Copy)
                        mm_ps = ppool.tile([P, G], f32, tag="mmps")
                        nc.tensor.matmul(
                            mm_ps[:], lhsT=aggT[:], rhs=rhs_s[r][:],
                            start=True, stop=True,
                        )
                        root = root1 if r <= 2 else root2
                        t_sb = wpool.tile([P, G], f32, tag="tsb")
                        nc.vector.scalar_tensor_tensor(
                            t_sb[:], mm_ps[:], dcol, root[:, b, :],
                            op0=Alu.mult, op1=Alu.add,
                        )

                        def store_y(yo, rr):
                            if b < NBH:
                                nc.sync.dma_start(
                                    out=agA[rr][b * P:(b + 1) * P, :], in_=yo[:]
                                )
                            else:
                                bb = b - NBH
                                nc.sync.dma_start(
                                    out=agB[rr][bb * P:(bb + 1) * P, :], in_=yo[:]
                                )

                        if r == 1:
                            yo = wpool.tile([P, G1], YDT[2], tag="yout")
                            nc.scalar.activation(yo[:], t_sb[:], Act.Relu, scale=dcol)
                            store_y(yo, 2)
                        elif r == 2:
                            out1 = wpool.tile([P, G1], f32, tag="out1")
                            nc.scalar.activation(out1[:], t_sb[:], Act.Relu)
                            hsum = wpool.tile([P, HID_C], f32, tag="hsum")
                            nc.vector.tensor_add(
                                hsum[:], out1[:, :HID_C], out1[:, HID_C:]
                            )
                            yo = wpool.tile([P, HID_C], f32, tag="yout2")
                            nc.scalar.activation(
                                yo[:], hsum[:], Act.Copy, scale=dinvh[:, b:b + 1]
                            )
                            store_y(yo, 3)
                            hT_ps = ppool.tile([HID_C, P], f32, tag="tps")
                            nc.tensor.transpose(hT_ps[:], hsum[:], ident[:])
                            hT = wpool.tile([HID_C, P], f32, tag="aggT")
                            nc.scalar.activation(hT[:], hT_ps[:], Act.Copy)
                            r2_ps = ppool.tile([P, G2], f32, tag="mmps")
                            nc.tensor.matmul(
                                r2_ps[:], lhsT=hT[:], rhs=rootw2_s[:],
                                start=True, stop=True,
                            )
                            nc.vector.tensor_add(root2[:, b, :], r2_ps[:], b2rep[:])
                        elif r == 3:
                            yo = wpool.tile([P, G2], f32, tag="yout")
                            nc.scalar.activation(yo[:], t_sb[:], Act.Relu, scale=dcol)
                            store_y(yo, 4)
                        else:
                            ofin = wpool.tile([P, G2], f32, tag="out1")
                            nc.scalar.activation(ofin[:], t_sb[:], Act.Relu)
                            msum = wpool.tile([P, OUT_C], f32, tag="hsum")
                            nc.vector.tensor_add(
                                msum[:], ofin[:, :OUT_C], ofin[:, OUT_C:]
                            )
                            yo = wpool.tile([P, OUT_C], f32, tag="yout")
                            nc.scalar.activation(yo[:], msum[:], Act.Copy, scale=0.5)
                            nc.sync.dma_start(
                                out=out_d[b * P:(b + 1) * P, :], in_=yo[:]
                            )
                    # fire AG_A as soon as the half-A output shard exists
                    if (b1 == NBH and r < 4 and r < max_round and not skip_ag
                            and not lite):
                        nc.gpsimd.collective_compute(
                            "AllGather", Alu.bypass, replica_groups=rg,
                            ins=[agA[r + 1].ap().opt()],
                            outs=[yA[r + 1].ap().opt()],
                        )
                if r < 4 and r < max_round and not skip_ag and not lite:
                    nc.gpsimd.collective_compute(
                        "AllGather", Alu.bypass, replica_groups=rg,
                        ins=[agB[r + 1].ap().opt()], outs=[yB[r + 1].ap().opt()],
                    )

    nc.compile()
    return nc


def build_in_maps(inputs: dict, cfg: Cfg, st: Struct) -> list:
    x = np.asarray(inputs["x"], dtype=np.float32)
    wmap = build_weight_inputs(inputs, cfg)
    in_maps = []
    for c in range(cfg.CORES):
        xs = np.zeros((cfg.NPC, cfg.IN_C), np.float32)
        mine = np.nonzero(st.pid // cfg.NPC == c)[0]
        loc = st.pid[mine] % cfg.NPC
        xs[loc] = x[mine]
        m = {
            "xs": xs,
            "idx16": np.ascontiguousarray(st.idx16[c]),
            "idx32": np.ascontiguousarray(st.idx32[c]),
        }
        m.update(wmap)
        in_maps.append(m)
    return in_maps


def assemble_output(results: list, cfg: Cfg, st: Struct) -> np.ndarray:
    full = np.concatenate(
        [np.asarray(results[c]["out"]) for c in range(cfg.CORES)], axis=0
    )
    return np.ascontiguousarray(full[st.pid]).astype(np.float32)


def kernel(**inputs) -> np.ndarray:
    from concourse.bass_utils import run_bass_kernel_spmd

    cfg = Cfg()
    st = build_structure(np.asarray(inputs["edge_index"]), cfg)
    nc = build_nc(cfg, st)
    in_maps = build_in_maps(inputs, cfg, st)
    res = run_bass_kernel_spmd(nc, in_maps, core_ids=list(range(cfg.CORES)))
    return assemble_output(res.results, cfg, st)


if __name__ == "__main__":
    pass
